# revision 23
# baseline (speedup 1.0000x reference)
"""Trainium2 Bass kernel for nn_CT_loss (data-parallel over batch, 8 cores).

Math (R is a general 3x3 matrix, not orthogonal):
  u   = A P0 + b0          A = R diag(e), b0 = t - 0.5 R e      (per batch)
  c_a = G_a . P0 + g0_a    G = R^T A,     g0 = R^T b0
  vt_a= (alpha_a qA + beta_a qB + h_a)/s_a   (qX = raw Q0 channels)
  y   = u - c_a vt_a ;  la = |s_a| sqrt(|y|^2)  on pixels where m_a=1
  loss = sum_a [sum(m_a) >= 3B] sum(la) / max(sum occmask, 1)

v3: mask compaction. The host gathers, per (batch, a) "granule", only the
~8.2k of 16.4k pixels with m_a=1 (plus zero padding to a fixed 8960) and
ships 5 compacted channels (x,y,z,qA,qB). The device never sees masks and
the host subtracts the (constant-input) zero-pad contribution from each
granule sum, replaying the device's bf16 rounding, then applies |s_a|,
the gating and the occmask normalization.

v3.2: per-granule QR rotation. |y| is invariant under orthogonal maps, so
rotate by Q^T where [alpha beta] = Q [[a1,b1],[0,b2],[0,0]]. Then
  vt'_0 = a1 qA + b1 qB + h'_0     (affine2)
  vt'_1 =         b2 qB + h'_1     (one tensor_scalar)
  vt'_2 =                 h'_2     (constant -> t_2 = c * h'_2, a TS)
and u' = Q^T A P0 + Q^T b0 costs the same as u. Saves one 3-channel
tensor add, one 3-channel mul and two ACT terms per granule.

Layout per core: 24 granules (8 batches x 3 a) x 5 partitions x 1792 px.
Per-granule scalars ride as per-partition columns of a constants tile.
Free-dim sums via accum_out; a 32x32 stream transpose packs the [128,1]
accumulator into 4 partition rows so the output DMA is 4 descriptors.
"""
import os
import sys

import numpy as np

for _p in ("/opt/trn_rl_repo",):
    if _p not in sys.path:
        sys.path.insert(0, _p)

import concourse.bass as bass
import concourse.bacc as bacc
import concourse.tile as tile
from concourse import mybir
from concourse.bass_utils import run_bass_kernel_spmd

from ml_dtypes import bfloat16

F32 = mybir.dt.float32
BF16 = mybir.dt.bfloat16
AF = mybir.ActivationFunctionType
OP = mybir.AluOpType

B, HW = 64, 128 * 128
NCORES, BPC = 8, 8
NG = BPC * 3          # granules per core
LPG = 5               # lanes (partitions) per granule
NP = NG * LPG         # 120 active partitions
FD = 1792             # pixels per lane
PL = LPG * FD         # 8960 padded pixels per granule

# a -> (Acol1, Acol2, qchA, qchB)
QCH = {0: (1, 2, 0, 1), 1: (0, 2, 2, 3), 2: (0, 1, 4, 5)}

# constants tile columns (per granule row)
KAX = 0    # A'[i,0] i=0..2
KAY = 3    # A'[i,1]
KAZ = 6    # A'[i,2]
KB0 = 9    # b0'_i
KG = 12    # G[a,0..2]
KG0 = 15   # g0_a
KA1 = 16   # a1 (qA scale, vt0)
KB1 = 17   # b1 (qB scale, vt0)
KH0 = 18   # h'_0
KB2 = 19   # b2 (qB scale, vt1)
KH1 = 20   # h'_1
KH2 = 21   # h'_2 (t2 = c * h'_2)
KZ = 22
NC2 = 23

_BUILT = None
LAST = None


def _bcast3(ap, n):
    """[P, FD] AP -> [P, n, FD] with step-0 middle dim."""
    return bass.AP(tensor=ap.tensor, offset=ap.offset,
                   ap=[ap.ap[0], [0, n], *ap.ap[1:]])


def _build_nc():
    nc = bacc.Bacc(None)
    xin = nc.dram_tensor("xin", [5, NP, FD], BF16, kind="ExternalInput")
    cst = nc.dram_tensor("cst", [NP, NC2], F32, kind="ExternalInput")
    outp = nc.dram_tensor("out", [4, 32], F32, kind="ExternalOutput")

    with tile.TileContext(nc) as tc:
        with tc.tile_pool(name="main", bufs=1) as pool:
            cst_t = pool.tile([NP, NC2], F32, tag="cst")
            nc.sync.dma_start(cst_t[:], cst[:])

            def cs(j):
                return cst_t[:, j:j + 1]

            acc = pool.tile([128, 32], F32, tag="acc")
            nc.vector.memset(acc[:], 0.0)

            # Warm the sqrt table set before real work.
            warm = pool.tile([NP, 1], BF16, tag="warm")
            nc.scalar.activation(warm[:], cst_t[:, KZ:KZ + 1], AF.Sqrt)

            # input channels: x split across BOTH rings so it lands
            # first; remaining channels balanced across rings
            xt = pool.tile([NP, 5, FD], BF16, tag="xt")
            nc.sync.dma_start(xt[:, 0, :], xin[0])      # x
            nc.scalar.dma_start(xt[:, 3, :], xin[3])    # qA (ACT-consumed)
            nc.sync.dma_start(xt[:, 4, :], xin[4])      # qB
            nc.scalar.dma_start(xt[:, 1, :], xin[1])    # y
            nc.sync.dma_start(xt[:, 2, :], xin[2])      # z
            xc, yc, zc, qa, qb = (xt[:, j, :] for j in range(5))

            t1 = pool.tile([NP, 3, FD], BF16, tag="t1")
            t2 = pool.tile([NP, 3, FD], BF16, tag="t2")
            t3 = pool.tile([NP, 3, FD], BF16, tag="t3")
            c1 = pool.tile([NP, FD], BF16, tag="c1")
            c2 = pool.tile([NP, FD], BF16, tag="c2")
            c3 = pool.tile([NP, FD], BF16, tag="c3")
            vt = pool.tile([NP, 2, FD], BF16, tag="vt")
            ta0 = pool.tile([NP, FD], BF16, tag="ta0")
            tb0 = pool.tile([NP, FD], BF16, tag="tb0")
            u3 = pool.tile([NP, 3, FD], BF16, tag="u3")
            tt = pool.tile([NP, 3, FD], BF16, tag="tt")

            # x terms (split across ACT and DVE)
            nc.scalar.activation(t1[:, 0, :], xc, AF.Identity,
                                 bias=cs(KB0), scale=cs(KAX))
            nc.scalar.activation(t1[:, 1, :], xc, AF.Identity,
                                 bias=cs(KB0 + 1), scale=cs(KAX + 1))
            nc.vector.tensor_scalar(t1[:, 2, :], xc, cs(KAX + 2),
                                    cs(KB0 + 2), op0=OP.mult, op1=OP.add)
            nc.vector.tensor_scalar(c1[:], xc, cs(KG), cs(KG0),
                                    op0=OP.mult, op1=OP.add)
            # qA / qB terms
            nc.scalar.activation(ta0[:], qa, AF.Identity,
                                 bias=cs(KH0), scale=cs(KA1))
            nc.vector.tensor_scalar_mul(tb0[:], qb, cs(KB1))
            nc.vector.tensor_scalar(vt[:, 1, :], qb, cs(KB2), cs(KH1),
                                    op0=OP.mult, op1=OP.add)
            nc.vector.tensor_add(vt[:, 0, :], ta0[:], tb0[:])
            # y terms
            for i in range(3):
                nc.vector.tensor_scalar_mul(t2[:, i, :], yc, cs(KAY + i))
            nc.vector.tensor_scalar_mul(c2[:], yc, cs(KG + 1))
            nc.vector.tensor_add(t1[:], t1[:], t2[:])
            # z terms
            for i in range(3):
                nc.vector.tensor_scalar_mul(t3[:, i, :], zc, cs(KAZ + i))
            nc.vector.tensor_scalar_mul(c3[:], zc, cs(KG + 2))
            NPC = 2
            PW = FD // NPC
            sq = pool.tile([NP, 3, FD], BF16, tag="sq")
            w = pool.tile([NP, 2, FD], BF16, tag="w")
            la = pool.tile([NP, FD], BF16, tag="la")
            # tail per FD-piece: finish c and u, then y/w/sqrt, so piece 0
            # streams while piece 1's inputs are still being combined.
            # channel 2 is y2 = u2 - h2*c, host-folded to a pure affine3
            # whose final add writes straight into tt[:,2,:].
            for p in range(NPC):
                s = slice(p * PW, (p + 1) * PW)
                nc.vector.tensor_add(c1[:, s], c1[:, s], c2[:, s])
                nc.vector.tensor_add(c1[:, s], c1[:, s], c3[:, s])
                nc.vector.tensor_add(u3[:, 0:2, s], t1[:, 0:2, s],
                                     t3[:, 0:2, s])
                nc.vector.tensor_add(tt[:, 2, s], t1[:, 2, s], t3[:, 2, s])
                nc.vector.tensor_mul(tt[:, 0:2, s], _bcast3(c1[:, s], 2),
                                     vt[:, :, s])
                yp = tt[:, :, s]
                nc.vector.tensor_sub(tt[:, 0:2, s], u3[:, 0:2, s],
                                     tt[:, 0:2, s])
                if p == 0:
                    nc.scalar.activation(sq[:, :, s], yp, AF.Square)
                else:
                    nc.vector.tensor_mul(sq[:, :, s], yp, yp)
                nc.vector.tensor_add(w[:, 0, s], sq[:, 0, s], sq[:, 1, s])
                nc.vector.tensor_add(w[:, 1, s], w[:, 0, s], sq[:, 2, s])
                nc.scalar.activation(la[:, s], w[:, 1, s], AF.Sqrt,
                                     accum_out=acc[:NP, p:p + 1])

            # fold piece accumulators, then pack the acc column into 4
            # partition rows so the out DMA is 4 descriptors
            nc.vector.tensor_add(acc[:, 0:1], acc[:, 0:1], acc[:, 1:2])
            accT = pool.tile([128, 32], F32, tag="accT")
            nc.vector.transpose(accT[:], acc[:])
            nc.sync.dma_start(
                outp[:],
                bass.AP(tensor=accT.tensor, offset=accT[:].offset,
                        ap=[[32 * accT[:].ap[0][0], 4], [1, 32]]))

    nc.compile()
    return nc


def get_nc():
    global _BUILT
    if _BUILT is None:
        _BUILT = _build_nc()
    return _BUILT


def _bf(v):
    """Round f64 -> bf16 -> f64 (device rounding replay)."""
    return np.asarray(v, np.float64).astype(bfloat16).astype(np.float64)


def host_constants(R, T, E):
    """Per-(b,a) granule constants [B, 3, NC2] f32, |s| [B,3], la_pad [B,3]."""
    Bn = R.shape[0]
    out = np.zeros((Bn, 3, NC2), np.float64)
    sabs = np.zeros((Bn, 3), np.float64)
    lapad = np.zeros((Bn, 3), np.float64)
    for b in range(Bn):
        Rb = R[b].astype(np.float64)
        tb = T[b].astype(np.float64)
        eb = E[b].astype(np.float64)
        A = Rb * eb[None, :]
        b0 = tb - 0.5 * (Rb @ eb)
        Gm = Rb.T @ A
        g0 = Rb.T @ b0
        s = Rb.T @ tb
        for a, (cc1, cc2, _, _) in QCH.items():
            sh = np.sign(s[a]) * max(abs(s[a]), 1e-12) if s[a] != 0 else 1e-12
            sabs[b, a] = abs(s[a])
            h = (tb - 0.5 * (A[:, cc1] + A[:, cc2])) / sh
            al = A[:, cc1] / sh
            be = A[:, cc2] / sh
            # rotate by Q^T: [al be] = Q [[a1,b1],[0,b2],[0,0]]
            Q, Rr = np.linalg.qr(np.stack([al, be], axis=1), mode="complete")
            a1, b1, b2 = Rr[0, 0], Rr[0, 1], Rr[1, 1]
            hp = Q.T @ h
            Ap = Q.T @ A
            b0p = Q.T @ b0
            row2 = Ap[2, :] - hp[2] * Gm[a]
            bias2 = b0p[2] - hp[2] * g0[a]
            out[b, a, KAX:KAX + 3] = [Ap[0, 0], Ap[1, 0], row2[0]]
            out[b, a, KAY:KAY + 3] = [Ap[0, 1], Ap[1, 1], row2[1]]
            out[b, a, KAZ:KAZ + 3] = [Ap[0, 2], Ap[1, 2], row2[2]]
            out[b, a, KB0:KB0 + 3] = [b0p[0], b0p[1], bias2]
            out[b, a, KG:KG + 3] = Gm[a]
            out[b, a, KG0] = g0[a]
            out[b, a, KA1] = a1
            out[b, a, KB1] = b1
            out[b, a, KH0] = hp[0]
            out[b, a, KB2] = b2
            out[b, a, KH1] = hp[1]
            out[b, a, KH2] = hp[2]
            # pad-pixel la with the device's bf16 rounding replayed
            c0 = _bf(g0[a])
            vt0 = np.array([_bf(hp[0]), _bf(hp[1])])
            t0 = np.array([_bf(c0 * vt0[0]), _bf(c0 * vt0[1])])
            y0 = _bf(_bf(b0p[:2]) - t0)
            y2 = _bf(bias2)
            sq0 = _bf(y0 * y0)
            w0 = _bf(_bf(sq0[0] + sq0[1]) + _bf(y2 * y2))
            lapad[b, a] = _bf(np.sqrt(w0))
    return out.astype(np.float32), sabs, lapad


def make_in_maps(P0, Q0, M, cst):
    in_maps = []
    npad = np.zeros((B, 3), np.int64)
    P0f = P0.reshape(B, 3, HW)
    Q0f = Q0.reshape(B, 6, HW)
    Mf = M.reshape(B, 3, HW)
    for k in range(NCORES):
        xin = np.zeros((5, NP, FD), np.float32)
        cst2 = np.zeros((NP, NC2), np.float32)
        for bi in range(BPC):
            b = k * BPC + bi
            for a in range(3):
                g = 3 * bi + a
                idx = np.flatnonzero(Mf[b, a])
                L = idx.size
                assert L <= PL, f"granule ({b},{a}) has {L} > {PL} pixels"
                npad[b, a] = PL - L
                _, _, qA, qB = QCH[a]
                rows = slice(LPG * g, LPG * g + LPG)
                for ci, src in enumerate((P0f[b, 0], P0f[b, 1], P0f[b, 2],
                                          Q0f[b, qA], Q0f[b, qB])):
                    buf = np.zeros(PL, np.float32)
                    buf[:L] = src[idx]
                    xin[ci, rows, :] = buf.reshape(LPG, FD)
                cst2[rows, :] = cst[b, a]
        in_maps.append({"xin": xin.astype(bfloat16),
                        "cst": np.ascontiguousarray(cst2)})
    return in_maps, npad


def kernel(pred_rots, pred_P0, pred_Q0, gt_occmask, roi_extent, pred_transes):
    global LAST
    R = np.asarray(pred_rots, np.float32)
    P0 = np.asarray(pred_P0, np.float32)
    Q0 = np.asarray(pred_Q0, np.float32)
    M = np.asarray(gt_occmask, np.float32)
    E = np.asarray(roi_extent, np.float32)
    T = np.asarray(pred_transes, np.float32)

    nc = get_nc()
    cst, sabs, lapad = host_constants(R, T, E)
    in_maps, npad = make_in_maps(P0, Q0, M, cst)
    trace = os.environ.get("KERNEL_TRACE", "0") == "1"
    LAST = run_bass_kernel_spmd(nc, in_maps, core_ids=list(range(NCORES)),
                                trace=trace)
    S_a = np.zeros(3, np.float64)
    for k, r in enumerate(LAST.results):
        o = r["out"].astype(np.float64).reshape(128)   # acc per partition
        gsum = o[:NP].reshape(NG, LPG).sum(axis=1)     # per-granule sums
        for bi in range(BPC):
            b = k * BPC + bi
            for a in range(3):
                s = gsum[3 * bi + a] - npad[b, a] * lapad[b, a]
                S_a[a] += s * sabs[b, a]
    M_a = M.reshape(B, 3, HW).sum(axis=(0, 2)).astype(np.float64)
    loss = sum(0.0 if M_a[a] < 3 * B else S_a[a] for a in range(3))
    total = max(M_a.sum(), 1.0)
    return np.asarray(np.float32(loss / total))


# revision 25
# speedup vs baseline: 1.0294x; 1.0294x over previous
"""Trainium2 Bass kernel for nn_CT_loss (data-parallel over batch, 8 cores).

Math (R is a general 3x3 matrix, not orthogonal):
  u   = A P0 + b0          A = R diag(e), b0 = t - 0.5 R e      (per batch)
  c_a = G_a . P0 + g0_a    G = R^T A,     g0 = R^T b0
  vt_a= (alpha_a qA + beta_a qB + h_a)/s_a   (qX = raw Q0 channels)
  y   = u - c_a vt_a ;  la = |s_a| sqrt(|y|^2)  on pixels where m_a=1
  loss = sum_a [sum(m_a) >= 3B] sum(la) / max(sum occmask, 1)

v3: mask compaction. The host gathers, per (batch, a) "granule", only the
~8.2k of 16.4k pixels with m_a=1 (plus zero padding to a fixed 8960) and
ships 5 compacted channels (x,y,z,qA,qB). The device never sees masks and
the host subtracts the (constant-input) zero-pad contribution from each
granule sum, replaying the device's bf16 rounding, then applies |s_a|,
the gating and the occmask normalization.

v3.2: per-granule QR rotation. |y| is invariant under orthogonal maps, so
rotate by Q^T where [alpha beta] = Q [[a1,b1],[0,b2],[0,0]]. Then
  vt'_0 = a1 qA + b1 qB + h'_0     (affine2)
  vt'_1 =         b2 qB + h'_1     (one tensor_scalar)
  vt'_2 =                 h'_2     (constant -> t_2 = c * h'_2, a TS)
and u' = Q^T A P0 + Q^T b0 costs the same as u. Saves one 3-channel
tensor add, one 3-channel mul and two ACT terms per granule.

Layout per core: 24 granules (8 batches x 3 a) x 5 partitions x 1792 px.
Per-granule scalars ride as per-partition columns of a constants tile.
Free-dim sums via accum_out; a 32x32 stream transpose packs the [128,1]
accumulator into 4 partition rows so the output DMA is 4 descriptors.
"""
import os
import sys

import numpy as np

for _p in ("/opt/trn_rl_repo",):
    if _p not in sys.path:
        sys.path.insert(0, _p)

import concourse.bass as bass
import concourse.bacc as bacc
import concourse.tile as tile
from concourse import mybir
from concourse.bass_utils import run_bass_kernel_spmd

from ml_dtypes import bfloat16

F32 = mybir.dt.float32
BF16 = mybir.dt.bfloat16
AF = mybir.ActivationFunctionType
OP = mybir.AluOpType

B, HW = 64, 128 * 128
NCORES, BPC = 8, 8
NG = BPC * 3          # granules per core
LPG = 5               # lanes (partitions) per granule
NP = NG * LPG         # 120 active partitions
FD = 1792             # pixels per lane
PL = LPG * FD         # 8960 padded pixels per granule

# a -> (Acol1, Acol2, qchA, qchB)
QCH = {0: (1, 2, 0, 1), 1: (0, 2, 2, 3), 2: (0, 1, 4, 5)}

# constants tile columns (per granule row)
KAX = 0    # A'[i,0] i=0..2
KAY = 3    # A'[i,1]
KAZ = 6    # A'[i,2]
KB0 = 9    # b0'_i
KG = 12    # G[a,0..2]
KG0 = 15   # g0_a
KA1 = 16   # a1 (qA scale, vt0)
KB1 = 17   # b1 (qB scale, vt0)
KH0 = 18   # h'_0
KB2 = 19   # b2 (qB scale, vt1)
KH1 = 20   # h'_1
KH2 = 21   # h'_2 (t2 = c * h'_2)
KZ = 22
NC2 = 23

_BUILT = None
LAST = None


def _bcast3(ap, n):
    """[P, FD] AP -> [P, n, FD] with step-0 middle dim."""
    return bass.AP(tensor=ap.tensor, offset=ap.offset,
                   ap=[ap.ap[0], [0, n], *ap.ap[1:]])


def _build_nc():
    nc = bacc.Bacc(None)
    xin = nc.dram_tensor("xin", [5, NP, FD], BF16, kind="ExternalInput")
    cst = nc.dram_tensor("cst", [NP, NC2], F32, kind="ExternalInput")
    outp = nc.dram_tensor("out", [4, 32], F32, kind="ExternalOutput")

    with tile.TileContext(nc) as tc:
        with tc.tile_pool(name="main", bufs=1) as pool:
            cst_t = pool.tile([NP, NC2], F32, tag="cst")
            nc.sync.dma_start(cst_t[:], cst[:])

            def cs(j):
                return cst_t[:, j:j + 1]

            acc = pool.tile([128, 32], F32, tag="acc")
            nc.vector.memset(acc[:], 0.0)

            # Warm the sqrt table set before real work.
            warm = pool.tile([NP, 1], BF16, tag="warm")
            nc.scalar.activation(warm[:], cst_t[:, KZ:KZ + 1], AF.Sqrt)

            # input channels: x split across BOTH rings so it lands
            # first; remaining channels balanced across rings
            xt = pool.tile([NP, 5, FD], BF16, tag="xt")
            nc.sync.dma_start(xt[:, 0, :], xin[0])      # x
            nc.scalar.dma_start(xt[:, 4, :], xin[4])    # qB
            nc.sync.dma_start(xt[:, 3, :], xin[3])      # qA
            nc.scalar.dma_start(xt[:, 2, :], xin[2])    # z
            nc.sync.dma_start(xt[:, 1, :], xin[1])      # y
            xc, yc, zc, qa, qb = (xt[:, j, :] for j in range(5))

            t1 = pool.tile([NP, 3, FD], BF16, tag="t1")
            t2 = pool.tile([NP, 3, FD], BF16, tag="t2")
            t3 = pool.tile([NP, 3, FD], BF16, tag="t3")
            c1 = pool.tile([NP, FD], BF16, tag="c1")
            c2 = pool.tile([NP, FD], BF16, tag="c2")
            c3 = pool.tile([NP, FD], BF16, tag="c3")
            vt = pool.tile([NP, 2, FD], BF16, tag="vt")
            ta0 = pool.tile([NP, FD], BF16, tag="ta0")
            tb0 = pool.tile([NP, FD], BF16, tag="tb0")
            u3 = pool.tile([NP, 3, FD], BF16, tag="u3")
            tt = pool.tile([NP, 3, FD], BF16, tag="tt")

            # x terms (split across ACT and DVE)
            nc.scalar.activation(t1[:, 0, :], xc, AF.Identity,
                                 bias=cs(KB0), scale=cs(KAX))
            nc.scalar.activation(t1[:, 1, :], xc, AF.Identity,
                                 bias=cs(KB0 + 1), scale=cs(KAX + 1))
            nc.vector.tensor_scalar(t1[:, 2, :], xc, cs(KAX + 2),
                                    cs(KB0 + 2), op0=OP.mult, op1=OP.add)
            nc.vector.tensor_scalar(c1[:], xc, cs(KG), cs(KG0),
                                    op0=OP.mult, op1=OP.add)
            # qA / qB terms
            nc.scalar.activation(ta0[:], qa, AF.Identity,
                                 bias=cs(KH0), scale=cs(KA1))
            nc.vector.tensor_scalar_mul(tb0[:], qb, cs(KB1))
            nc.vector.tensor_scalar(vt[:, 1, :], qb, cs(KB2), cs(KH1),
                                    op0=OP.mult, op1=OP.add)
            nc.vector.tensor_add(vt[:, 0, :], ta0[:], tb0[:])
            # y terms
            for i in range(3):
                nc.vector.tensor_scalar_mul(t2[:, i, :], yc, cs(KAY + i))
            nc.vector.tensor_scalar_mul(c2[:], yc, cs(KG + 1))
            nc.vector.tensor_add(t1[:], t1[:], t2[:])
            # z terms
            for i in range(3):
                nc.vector.tensor_scalar_mul(t3[:, i, :], zc, cs(KAZ + i))
            nc.vector.tensor_scalar_mul(c3[:], zc, cs(KG + 2))
            NPC = 2
            PW = FD // NPC
            sq = pool.tile([NP, 3, FD], BF16, tag="sq")
            w = pool.tile([NP, 2, FD], BF16, tag="w")
            la = pool.tile([NP, FD], BF16, tag="la")
            # tail per FD-piece: finish c and u, then y/w/sqrt, so piece 0
            # streams while piece 1's inputs are still being combined.
            # channel 2 is y2 = u2 - h2*c, host-folded to a pure affine3
            # whose final add writes straight into tt[:,2,:].
            for p in range(NPC):
                s = slice(p * PW, (p + 1) * PW)
                nc.vector.tensor_add(c1[:, s], c1[:, s], c2[:, s])
                nc.vector.tensor_add(c1[:, s], c1[:, s], c3[:, s])
                nc.vector.tensor_add(u3[:, :, s], t1[:, :, s], t3[:, :, s])
                nc.vector.tensor_mul(tt[:, 0:2, s], _bcast3(c1[:, s], 2),
                                     vt[:, :, s])
                nc.vector.tensor_sub(tt[:, 0:2, s], u3[:, 0:2, s],
                                     tt[:, 0:2, s])
                if p == 0:
                    nc.scalar.activation(sq[:, 0:2, s], tt[:, 0:2, s],
                                         AF.Square)
                    nc.scalar.activation(sq[:, 2, s], u3[:, 2, s], AF.Square)
                else:
                    nc.vector.tensor_mul(sq[:, 0:2, s], tt[:, 0:2, s],
                                         tt[:, 0:2, s])
                    nc.vector.tensor_mul(sq[:, 2, s], u3[:, 2, s],
                                         u3[:, 2, s])
                nc.vector.tensor_add(w[:, 0, s], sq[:, 0, s], sq[:, 1, s])
                nc.vector.tensor_add(w[:, 1, s], w[:, 0, s], sq[:, 2, s])
                nc.scalar.activation(la[:, s], w[:, 1, s], AF.Sqrt,
                                     accum_out=acc[:NP, p:p + 1])

            # fold piece accumulators, then pack the acc column into 4
            # partition rows so the out DMA is 4 descriptors
            nc.vector.tensor_add(acc[:, 0:1], acc[:, 0:1], acc[:, 1:2])
            accT = pool.tile([128, 32], F32, tag="accT")
            nc.vector.transpose(accT[:], acc[:])
            nc.sync.dma_start(
                outp[:],
                bass.AP(tensor=accT.tensor, offset=accT[:].offset,
                        ap=[[32 * accT[:].ap[0][0], 4], [1, 32]]))

    nc.compile()
    return nc


def get_nc():
    global _BUILT
    if _BUILT is None:
        _BUILT = _build_nc()
    return _BUILT


def _bf(v):
    """Round f64 -> bf16 -> f64 (device rounding replay)."""
    return np.asarray(v, np.float64).astype(bfloat16).astype(np.float64)


def host_constants(R, T, E):
    """Per-(b,a) granule constants [B, 3, NC2] f32, |s| [B,3], la_pad [B,3]."""
    Bn = R.shape[0]
    out = np.zeros((Bn, 3, NC2), np.float64)
    sabs = np.zeros((Bn, 3), np.float64)
    lapad = np.zeros((Bn, 3), np.float64)
    for b in range(Bn):
        Rb = R[b].astype(np.float64)
        tb = T[b].astype(np.float64)
        eb = E[b].astype(np.float64)
        A = Rb * eb[None, :]
        b0 = tb - 0.5 * (Rb @ eb)
        Gm = Rb.T @ A
        g0 = Rb.T @ b0
        s = Rb.T @ tb
        for a, (cc1, cc2, _, _) in QCH.items():
            sh = np.sign(s[a]) * max(abs(s[a]), 1e-12) if s[a] != 0 else 1e-12
            sabs[b, a] = abs(s[a])
            h = (tb - 0.5 * (A[:, cc1] + A[:, cc2])) / sh
            al = A[:, cc1] / sh
            be = A[:, cc2] / sh
            # rotate by Q^T: [al be] = Q [[a1,b1],[0,b2],[0,0]]
            Q, Rr = np.linalg.qr(np.stack([al, be], axis=1), mode="complete")
            a1, b1, b2 = Rr[0, 0], Rr[0, 1], Rr[1, 1]
            hp = Q.T @ h
            Ap = Q.T @ A
            b0p = Q.T @ b0
            row2 = Ap[2, :] - hp[2] * Gm[a]
            bias2 = b0p[2] - hp[2] * g0[a]
            out[b, a, KAX:KAX + 3] = [Ap[0, 0], Ap[1, 0], row2[0]]
            out[b, a, KAY:KAY + 3] = [Ap[0, 1], Ap[1, 1], row2[1]]
            out[b, a, KAZ:KAZ + 3] = [Ap[0, 2], Ap[1, 2], row2[2]]
            out[b, a, KB0:KB0 + 3] = [b0p[0], b0p[1], bias2]
            out[b, a, KG:KG + 3] = Gm[a]
            out[b, a, KG0] = g0[a]
            out[b, a, KA1] = a1
            out[b, a, KB1] = b1
            out[b, a, KH0] = hp[0]
            out[b, a, KB2] = b2
            out[b, a, KH1] = hp[1]
            out[b, a, KH2] = hp[2]
            # pad-pixel la with the device's bf16 rounding replayed
            c0 = _bf(g0[a])
            vt0 = np.array([_bf(hp[0]), _bf(hp[1])])
            t0 = np.array([_bf(c0 * vt0[0]), _bf(c0 * vt0[1])])
            y0 = _bf(_bf(b0p[:2]) - t0)
            y2 = _bf(bias2)
            sq0 = _bf(y0 * y0)
            w0 = _bf(_bf(sq0[0] + sq0[1]) + _bf(y2 * y2))
            lapad[b, a] = _bf(np.sqrt(w0))
    return out.astype(np.float32), sabs, lapad


def make_in_maps(P0, Q0, M, cst):
    in_maps = []
    npad = np.zeros((B, 3), np.int64)
    P0f = P0.reshape(B, 3, HW)
    Q0f = Q0.reshape(B, 6, HW)
    Mf = M.reshape(B, 3, HW)
    for k in range(NCORES):
        xin = np.zeros((5, NP, FD), np.float32)
        cst2 = np.zeros((NP, NC2), np.float32)
        for bi in range(BPC):
            b = k * BPC + bi
            for a in range(3):
                g = 3 * bi + a
                idx = np.flatnonzero(Mf[b, a])
                L = idx.size
                assert L <= PL, f"granule ({b},{a}) has {L} > {PL} pixels"
                npad[b, a] = PL - L
                _, _, qA, qB = QCH[a]
                rows = slice(LPG * g, LPG * g + LPG)
                for ci, src in enumerate((P0f[b, 0], P0f[b, 1], P0f[b, 2],
                                          Q0f[b, qA], Q0f[b, qB])):
                    buf = np.zeros(PL, np.float32)
                    buf[:L] = src[idx]
                    xin[ci, rows, :] = buf.reshape(LPG, FD)
                cst2[rows, :] = cst[b, a]
        in_maps.append({"xin": xin.astype(bfloat16),
                        "cst": np.ascontiguousarray(cst2)})
    return in_maps, npad


def kernel(pred_rots, pred_P0, pred_Q0, gt_occmask, roi_extent, pred_transes):
    global LAST
    R = np.asarray(pred_rots, np.float32)
    P0 = np.asarray(pred_P0, np.float32)
    Q0 = np.asarray(pred_Q0, np.float32)
    M = np.asarray(gt_occmask, np.float32)
    E = np.asarray(roi_extent, np.float32)
    T = np.asarray(pred_transes, np.float32)

    nc = get_nc()
    cst, sabs, lapad = host_constants(R, T, E)
    in_maps, npad = make_in_maps(P0, Q0, M, cst)
    trace = os.environ.get("KERNEL_TRACE", "0") == "1"
    LAST = run_bass_kernel_spmd(nc, in_maps, core_ids=list(range(NCORES)),
                                trace=trace)
    S_a = np.zeros(3, np.float64)
    for k, r in enumerate(LAST.results):
        o = r["out"].astype(np.float64).reshape(128)   # acc per partition
        gsum = o[:NP].reshape(NG, LPG).sum(axis=1)     # per-granule sums
        for bi in range(BPC):
            b = k * BPC + bi
            for a in range(3):
                s = gsum[3 * bi + a] - npad[b, a] * lapad[b, a]
                S_a[a] += s * sabs[b, a]
    M_a = M.reshape(B, 3, HW).sum(axis=(0, 2)).astype(np.float64)
    loss = sum(0.0 if M_a[a] < 3 * B else S_a[a] for a in range(3))
    total = max(M_a.sum(), 1.0)
    return np.asarray(np.float32(loss / total))
